# revision 32
# baseline (speedup 1.0000x reference)
"""DeepseekV3 top-k router kernel for Trainium2 (8 NeuronCores, SPMD over tokens).

Strategy: data-parallel over the token dim (16384 tokens -> 2048/core).
Per core: router GEMM as a single fp16 matmul per 128-K chunk (fp32 PSUM
accumulate; the 2e-2 error gate leaves plenty of room for fp16 rounding),
sigmoid on ScalarE, group-limited top-8 selection split across VectorE (DVE)
and the otherwise-idle Pool engine. Top-8 weights are reconstructed as
m8 - bias[idx] via an equality-match gather of the bias table, avoiding a
separate uncorrected-score tensor.
"""

import numpy as np

import concourse.bass as bass
import concourse.mybir as mybir
import concourse.tile as tile
from concourse.bass_utils import run_bass_kernel_spmd

# Problem constants (hardcoded per contract).
TOP_K = 8
N_EXPERTS = 256
N_GROUP = 8
PER_GROUP = N_EXPERTS // N_GROUP  # 32
TOPK_GROUP = 4
ROUTED_SCALING = 2.5
HIDDEN = 7168
TOKENS = 16384
N_CORES = 8
P = 128  # partitions / tokens per tile
KC = HIDDEN // P  # 56 contraction chunks
NEG_BIG = -1.0e30
FP16_MIN_NORMAL = 6.104e-5
KEY_C = 0.0625  # offset keeping (masked_scorr + C) positive for u32 keys
SEL_SCALE = 7021.0  # 13-bit selection grid: (1.104 + C) * 7021 < 8192
RAW_SCALE = 262048.0  # 8189 * 32: payload pre-shifted into bits [5, 18)


def _int_imm(inst, dtype):
    """Retype float immediates of a lowered instruction as integers (walrus
    requires bitvec-op immediates to be integer-typed and match src/dst)."""
    for arg in inst.ins.ins:
        if isinstance(arg, mybir.ImmediateValue):
            arg.dtype = dtype
            arg.value = int(arg.value)
    return inst

f32 = mybir.dt.float32
f16 = mybir.dt.float16
u32 = mybir.dt.uint32
i32 = mybir.dt.int32

# walrus in this toolchain rejects more than one sync-wait per instruction.
# Post-pass: move excess waits onto same-engine NOPs inserted just before the
# offending instruction (engine stalls on the NOPs first — semantics preserved).
_MAX_WAITS = 1


def _split_excess_waits(nc, max_waits=_MAX_WAITS):
    all_bbs = [bb for fn in nc.m.functions for bb in fn.blocks]
    pre_by_name = {}
    appended = set()
    for bb in all_bbs:
        for inst in bb.instructions:
            si = inst.sync_info
            if si is None:
                continue
            waits = list(si.on_wait or [])
            if len(waits) <= max_waits:
                continue
            if inst.engine not in nc.engines:
                continue
            eng = nc.engines[inst.engine]
            n_extra = len(waits) - max_waits
            pre = []
            for j in range(0, n_extra, max_waits):
                nb = eng.nop(nofuse=True)
                nb.ins.sync_info = mybir.SyncInfo(
                    on_wait=waits[j : j + max_waits], on_update=[]
                )
                pre.append(nb.ins)
                appended.add(nb.ins.name)
            si.on_wait = waits[n_extra:]
            inst.sync_info = si
            pre_by_name[inst.name] = pre
    if not pre_by_name:
        return
    for bb in all_bbs:
        rebuilt = []
        changed = False
        for inst in bb.instructions:
            if inst.name in appended:
                changed = True
                continue
            if inst.name in pre_by_name:
                rebuilt.extend(pre_by_name[inst.name])
                changed = True
            rebuilt.append(inst)
        if changed:
            bb.instructions = rebuilt


def build_program(tokens_per_core: int, debug: bool = False):
    """Build the single-core Bass program (same program runs SPMD on all cores)."""
    ntiles = tokens_per_core // P
    nc = bass.Bass("TRN2", target_bir_lowering=False, debug=False)
    dbg = {}
    if debug:
        for nm in ["d_scores", "d_mscorr", "d_selq", "d_rawq", "d_key"]:
            dt_ = f32 if nm in ("d_scores", "d_mscorr") else u32
            dbg[nm] = nc.dram_tensor(nm, [P, N_EXPERTS], dt_, kind="ExternalOutput").ap()
        dbg["d_m8k"] = nc.dram_tensor("d_m8k", [P, TOP_K], u32, kind="ExternalOutput").ap()
        dbg["d_w8u"] = nc.dram_tensor("d_w8u", [P, TOP_K], u32, kind="ExternalOutput").ap()

    # Host-prepared layouts (see prep_inputs):
    #  xt [ntiles, 128(p), 56(c), 128(t)] f16 ; wt [128(p), 56(c), 256(e)] f16
    #  bb [128, 256] f32 (bias row-broadcast)
    xt = nc.dram_tensor("xt", [ntiles, P, KC, P], f16, kind="ExternalInput").ap()
    wt = nc.dram_tensor("wt", [P, KC, N_EXPERTS], f16, kind="ExternalInput").ap()
    bb = nc.dram_tensor("bb", [P, N_EXPERTS], f32, kind="ExternalInput").ap()
    oi = nc.dram_tensor("oi", [tokens_per_core, TOP_K], i32, kind="ExternalOutput").ap()
    ow = nc.dram_tensor("ow", [tokens_per_core, TOP_K], f32, kind="ExternalOutput").ap()

    with tile.TileContext(nc) as tc:
        with (
            tc.tile_pool(name="wpool", bufs=1) as wpool,
            tc.tile_pool(name="xpool", bufs=5) as xpool,
            tc.tile_pool(name="psum", bufs=2, space="PSUM") as psum_pool,
            tc.tile_pool(name="spool", bufs=2) as spool,
            tc.tile_pool(name="small", bufs=2) as small,
            tc.tile_pool(name="opool", bufs=1) as opool,
        ):
            # PE warmup: ~40 matmuls on a zeroed scratch region while the
            # first DMAs land, so the frequency ramp happens off-path.
            warm_x = wpool.tile([P, P], f16, tag="warmx")
            nc.vector.memset(warm_x[:], 0.0)
            warm_w = wpool.tile([P, N_EXPERTS], f16, tag="warmw")
            nc.vector.memset(warm_w[:], 0.0)
            warm_ps = psum_pool.tile([P, N_EXPERTS], f32, tag="warmps")
            n_warm = 28
            for wu in range(n_warm):
                nc.tensor.matmul(
                    warm_ps[:],
                    warm_x[:],
                    warm_w[:],
                    start=(wu == 0),
                    stop=(wu == n_warm - 1),
                )

            # Weights stream first on BOTH rings concurrently (all resident by
            # ~6.5us), ahead of the x halves, so the GEMM never weight-stalls.
            w_pieces = [(0, 28, nc.scalar), (28, 28, nc.sync)]
            wt_flat = wt.rearrange("p c e -> p (c e)")
            w_tiles = []  # (start_chunk, n_chunks, tile)
            for ws_i, (cc0, n_cc, ring) in enumerate(w_pieces):
                wtile = wpool.tile([P, n_cc * N_EXPERTS], f16, tag=f"w{ws_i}")
                ring.dma_start(
                    wtile[:],
                    wt_flat[:, cc0 * N_EXPERTS : (cc0 + n_cc) * N_EXPERTS],
                )
                w_tiles.append((cc0, n_cc, wtile))
            bias_sb = wpool.tile([P, N_EXPERTS], f32)
            nc.sync.dma_start(bias_sb[:], bb)
            oi_sb = opool.tile([P, ntiles * TOP_K], u32)
            ow_sb = opool.tile([P, ntiles * TOP_K], f32)

            def wsl(cc):  # weight AP for chunk cc
                for start, n_cc, wtile in w_tiles:
                    if start <= cc < start + n_cc:
                        return wtile[:, bass.ts(cc - start, N_EXPERTS)]
                raise AssertionError(cc)

            oi_dram = oi.rearrange("(t p) k -> p t k", p=P)
            ow_dram = ow.rearrange("(t p) k -> p t k", p=P)

            half = (KC // 2) * P
            for tt in range(ntiles):
                # Load hidden tile (contiguous, 1.84 MB): halves on two rings.
                # The last tile is split finer so its GEMM can finish sooner
                # after the final bytes land.
                x_tile = xpool.tile([P, KC * P], f16)
                x_src = xt[tt].rearrange("p c t -> p (c t)")
                nc.scalar.dma_start(x_tile[:, :half], x_src[:, :half])
                nc.sync.dma_start(x_tile[:, half:], x_src[:, half:])

                # Router GEMM: logits[128t, 256e] accumulated over 56 K-chunks.
                ps = psum_pool.tile([P, N_EXPERTS], f32, tag="ps")
                for cc in range(KC):
                    nc.tensor.matmul(
                        ps[:],
                        x_tile[:, bass.ts(cc, P)],
                        wsl(cc),
                        start=(cc == 0),
                        stop=(cc == KC - 1),
                    )

                # scores = sigmoid(logits)
                scores = spool.tile([P, N_EXPERTS], f32, tag="scores")
                nc.scalar.activation(
                    scores[:], ps[:], mybir.ActivationFunctionType.Sigmoid
                )
                # corrected scores for selection
                scorr = spool.tile([P, N_EXPERTS], f32, tag="scorr")
                nc.vector.tensor_tensor(
                    scorr[:], scores[:], bias_sb[:], op=mybir.AluOpType.add
                )

                s3 = scorr[:].rearrange("p (g e) -> p g e", g=N_GROUP)
                # top-2-sum per group of 32 via one max8 per group
                g2 = small.tile([P, N_GROUP * 8], f32, tag="g2")
                for g in range(N_GROUP):
                    nc.vector.max(
                        g2[:, 8 * g : 8 * g + 8],
                        scorr[:, PER_GROUP * g : PER_GROUP * (g + 1)],
                    )
                g2v = g2[:].rearrange("p (g k) -> p g k", k=8)
                gs = small.tile([P, N_GROUP], f32, tag="gs")
                nc.vector.tensor_tensor(
                    gs[:].unsqueeze(2),
                    g2v[:, :, 0:1],
                    g2v[:, :, 1:2],
                    op=mybir.AluOpType.add,
                )

                # top-4 groups -> 0/1 expert mask (f32)
                g8 = small.tile([P, 8], f32, tag="g8")
                nc.vector.max(g8[:], gs[:])
                gmask = small.tile([P, N_GROUP], f32, tag="gmask")
                nc.vector.tensor_scalar(
                    gmask[:],
                    gs[:],
                    g8[:, TOPK_GROUP - 1 : TOPK_GROUP],
                    None,
                    op0=mybir.AluOpType.is_ge,
                )
                gmb = gmask[:].unsqueeze(2).broadcast_to([P, N_GROUP, PER_GROUP])

                # Packed u32 ranking keys:
                #   key = (u32((masked_scorr + C)*2^17) << 13) | u32(masked_score*8191)
                # Group-masked entries get exactly key(scorr=0, raw=0), matching
                # the reference's zero-fill. Ranking (high bits) follows the
                # corrected score; the low 13 bits carry the raw-score payload
                # so top-8 values + weights come out of one max8/max_index pair.
                mscorr = spool.tile([P, N_EXPERTS], f32, tag="mscorr")
                nc.vector.tensor_tensor(
                    mscorr[:].rearrange("p (g e) -> p g e", g=N_GROUP),
                    s3,
                    gmb,
                    op=mybir.AluOpType.mult,
                )
                selq = spool.tile([P, N_EXPERTS], u32, tag="selq")
                nc.vector.tensor_scalar(
                    selq[:],
                    mscorr[:],
                    KEY_C,
                    SEL_SCALE,
                    op0=mybir.AluOpType.add,
                    op1=mybir.AluOpType.mult,
                )
                # Raw payload needs no group masking: masked-out keys stay
                # strictly below any selected key (selq dominates). Bits [0,5)
                # are guard bits: max8 reports values rounded to f32's 24-bit
                # mantissa (ulp <= 128 at 2^31), so the payload in bits [5,18)
                # is perturbed by at most +-2.
                rawq = spool.tile([P, N_EXPERTS], u32, tag="rawq")
                nc.vector.tensor_scalar(
                    rawq[:], scores[:], RAW_SCALE, None, op0=mybir.AluOpType.mult
                )
                key = spool.tile([P, N_EXPERTS], u32, tag="key")
                kinst = nc.vector.scalar_tensor_tensor(
                    key[:],
                    in0=selq[:],
                    scalar=18,
                    in1=rawq[:],
                    op0=mybir.AluOpType.logical_shift_left,
                    op1=mybir.AluOpType.bitwise_or,
                )
                _int_imm(kinst, u32)

                # top-8 keys: indices straight to the output tile, weights from
                # the low 13 bits (the 1/8191 scale cancels in normalization).
                m8k = small.tile([P, TOP_K], u32, tag="m8k")
                nc.vector.max(m8k[:], key[:])
                nc.vector.max_index(oi_sb[:, bass.ts(tt, TOP_K)], m8k[:], key[:])
                w8u = small.tile([P, TOP_K], u32, tag="w8u")
                winst = nc.vector.tensor_scalar(
                    w8u[:],
                    m8k[:],
                    14,
                    19,
                    op0=mybir.AluOpType.logical_shift_left,
                    op1=mybir.AluOpType.logical_shift_right,
                )
                _int_imm(winst, u32)
                w8f = small.tile([P, TOP_K], f32, tag="w8f")
                nc.vector.tensor_copy(w8f[:], w8u[:])
                wsum = small.tile([P, 1], f32, tag="wsum")
                nc.vector.tensor_reduce(
                    wsum[:], w8f[:], axis=mybir.AxisListType.X, op=mybir.AluOpType.add
                )
                winv = small.tile([P, 1], f32, tag="winv")
                nc.vector.reciprocal(winv[:], wsum[:])
                if debug and tt == 2:
                    nc.sync.dma_start(dbg["d_scores"], scores[:])
                    nc.sync.dma_start(dbg["d_mscorr"], mscorr[:])
                    nc.sync.dma_start(dbg["d_selq"], selq[:])
                    nc.sync.dma_start(dbg["d_rawq"], rawq[:])
                    nc.sync.dma_start(dbg["d_key"], key[:])
                    nc.sync.dma_start(dbg["d_m8k"], m8k[:])
                    nc.sync.dma_start(dbg["d_w8u"], w8u[:])
                nc.vector.tensor_scalar(
                    ow_sb[:, bass.ts(tt, TOP_K)],
                    w8f[:],
                    winv[:, 0:1],
                    float(ROUTED_SCALING),
                    op0=mybir.AluOpType.mult,
                    op1=mybir.AluOpType.mult,
                )
                # Store this tile's outputs (token-major [tokens, 8] in DRAM)
                # so the program tail only waits on the last tile's store.
                nc.sync.dma_start(
                    oi_dram[:, tt : tt + 1, :],
                    oi_sb[:, bass.ts(tt, TOP_K)]
                    .rearrange("p (t k) -> p t k", k=TOP_K)
                    .bitcast(i32),
                )
                nc.sync.dma_start(
                    ow_dram[:, tt : tt + 1, :],
                    ow_sb[:, bass.ts(tt, TOP_K)].rearrange(
                        "p (t k) -> p t k", k=TOP_K
                    ),
                )

    _split_excess_waits(nc)
    return nc


def _fp16_ftz(a32):
    """fp32 -> fp16 with subnormals flushed to zero (matches PE behavior)."""
    h = a32.astype(np.float16)
    h[np.abs(h) < FP16_MIN_NORMAL] = np.float16(0.0)
    return h


def prep_inputs(hidden_states, weight, e_score_correction_bias, n_cores=N_CORES):
    """Host-side shard + re-layout. Returns per-core input maps."""
    hidden_states = np.ascontiguousarray(hidden_states, dtype=np.float32)
    weight = np.ascontiguousarray(weight, dtype=np.float32)
    bias = np.asarray(e_score_correction_bias, dtype=np.float32)

    tokens = hidden_states.shape[0]
    ntiles_total = tokens // P
    tiles_per_core = ntiles_total // n_cores

    bb = np.ascontiguousarray(np.broadcast_to(bias, (P, N_EXPERTS)))

    # [T, H] -> [ntiles, t, c, p] view -> [ntiles, p, c, t]
    xh = _fp16_ftz(hidden_states)
    xt_all = xh.reshape(ntiles_total, P, KC, P).transpose(0, 3, 2, 1)
    wh = _fp16_ftz(weight)
    wt = np.ascontiguousarray(wh.T).reshape(KC, P, N_EXPERTS).transpose(1, 0, 2)
    wt = np.ascontiguousarray(wt)  # [p, c, e]

    in_maps = []
    for c in range(n_cores):
        xt_core = np.ascontiguousarray(
            xt_all[c * tiles_per_core : (c + 1) * tiles_per_core]
        )
        in_maps.append({"xt": xt_core, "wt": wt, "bb": bb})
    return in_maps, tiles_per_core * P


_PROGRAM_CACHE = {}


def run(hidden_states, weight, e_score_correction_bias, trace=False):
    in_maps, tokens_per_core = prep_inputs(
        hidden_states, weight, e_score_correction_bias
    )
    if tokens_per_core not in _PROGRAM_CACHE:
        _PROGRAM_CACHE[tokens_per_core] = build_program(tokens_per_core)
    nc = _PROGRAM_CACHE[tokens_per_core]
    res = run_bass_kernel_spmd(nc, in_maps, list(range(N_CORES)), trace=trace)
    idx = np.concatenate([res.results[i]["oi"] for i in range(N_CORES)], axis=0)
    wts = np.concatenate([res.results[i]["ow"] for i in range(N_CORES)], axis=0)
    return (idx, wts), res


def kernel(hidden_states, weight, e_score_correction_bias):
    (idx, wts), _ = run(hidden_states, weight, e_score_correction_bias)
    return idx.astype(np.int32), wts.astype(np.float32)


# revision 33
# speedup vs baseline: 1.1685x; 1.1685x over previous
"""DeepseekV3 top-k router kernel for Trainium2 (8 NeuronCores, SPMD over tokens).

Strategy: data-parallel over the token dim (16384 tokens -> 2048/core).
Per core: router GEMM as a single fp16 matmul per 128-K chunk (fp32 PSUM
accumulate; the 2e-2 error gate leaves plenty of room for fp16 rounding),
sigmoid on ScalarE. Selection packs (quantized corrected score | raw-score
payload) into u32 keys so one max8/max_index pair yields both the top-8
indices and weights; group top-2 sums come from per-group max8.
GEMM is issued in two phases per tile (first/second half of K) with phase B
lagging one tile, so PE always has scalar-ring data to chew while the sync
ring catches up.
"""

import numpy as np

import concourse.bass as bass
import concourse.mybir as mybir
import concourse.tile as tile
from concourse.bass_utils import run_bass_kernel_spmd

# Problem constants (hardcoded per contract).
TOP_K = 8
N_EXPERTS = 256
N_GROUP = 8
PER_GROUP = N_EXPERTS // N_GROUP  # 32
TOPK_GROUP = 4
ROUTED_SCALING = 2.5
HIDDEN = 7168
TOKENS = 16384
N_CORES = 8
P = 128  # partitions / tokens per tile
KC = HIDDEN // P  # 56 contraction chunks
FP16_MIN_NORMAL = 6.104e-5
KEY_C = 0.0625  # offset keeping (masked_scorr + C) positive for u32 keys
SEL_SCALE = 7021.0  # 13-bit selection grid: (1.104 + C) * 7021 < 8192
RAW_SCALE = 262048.0  # 8189 * 32: payload pre-shifted into bits [5, 18)

f32 = mybir.dt.float32
f16 = mybir.dt.float16
u32 = mybir.dt.uint32
i32 = mybir.dt.int32


def _int_imm(inst, dtype):
    """Retype float immediates of a lowered instruction as integers (walrus
    requires bitvec-op immediates to be integer-typed and match src/dst)."""
    for arg in inst.ins.ins:
        if isinstance(arg, mybir.ImmediateValue):
            arg.dtype = dtype
            arg.value = int(arg.value)
    return inst


# walrus in this toolchain rejects more than one sync-wait per instruction.
# Post-pass: move excess waits onto same-engine NOPs inserted just before the
# offending instruction (engine stalls on the NOPs first — semantics preserved).
_MAX_WAITS = 1


def _split_excess_waits(nc, max_waits=_MAX_WAITS):
    all_bbs = [bb for fn in nc.m.functions for bb in fn.blocks]
    pre_by_name = {}
    appended = set()
    for bb in all_bbs:
        for inst in bb.instructions:
            si = inst.sync_info
            if si is None:
                continue
            waits = list(si.on_wait or [])
            if len(waits) <= max_waits:
                continue
            if inst.engine not in nc.engines:
                continue
            eng = nc.engines[inst.engine]
            n_extra = len(waits) - max_waits
            pre = []
            for j in range(0, n_extra, max_waits):
                nb = eng.nop(nofuse=True)
                nb.ins.sync_info = mybir.SyncInfo(
                    on_wait=waits[j : j + max_waits], on_update=[]
                )
                pre.append(nb.ins)
                appended.add(nb.ins.name)
            si.on_wait = waits[n_extra:]
            inst.sync_info = si
            pre_by_name[inst.name] = pre
    if not pre_by_name:
        return
    for bb in all_bbs:
        rebuilt = []
        changed = False
        for inst in bb.instructions:
            if inst.name in appended:
                changed = True
                continue
            if inst.name in pre_by_name:
                rebuilt.extend(pre_by_name[inst.name])
                changed = True
            rebuilt.append(inst)
        if changed:
            bb.instructions = rebuilt


def build_program(tokens_per_core: int):
    """Build the single-core Bass program (same program runs SPMD on all cores)."""
    ntiles = tokens_per_core // P
    nc = bass.Bass("TRN2", target_bir_lowering=False, debug=False)

    # Host-prepared layouts (see prep_inputs):
    #  xt [ntiles, 128(p), 56(c), 128(t)] f16 ; wt [128(p), 56(c), 256(e)] f16
    #  bb [128, 256] f32 (bias row-broadcast)
    xt = nc.dram_tensor("xt", [ntiles, P, KC, P], f16, kind="ExternalInput").ap()
    wt = nc.dram_tensor("wt", [P, KC, N_EXPERTS], f16, kind="ExternalInput").ap()
    bb = nc.dram_tensor("bb", [P, N_EXPERTS], f32, kind="ExternalInput").ap()
    oi = nc.dram_tensor("oi", [tokens_per_core, TOP_K], i32, kind="ExternalOutput").ap()
    ow = nc.dram_tensor("ow", [tokens_per_core, TOP_K], f32, kind="ExternalOutput").ap()

    with tile.TileContext(nc) as tc:
        with (
            tc.tile_pool(name="wpool", bufs=1) as wpool,
            tc.tile_pool(name="xpool", bufs=5) as xpool,
            tc.tile_pool(name="psum", bufs=3, space="PSUM") as psum_pool,
            tc.tile_pool(name="spool", bufs=2) as spool,
            tc.tile_pool(name="small", bufs=2) as small,
            tc.tile_pool(name="opool", bufs=1) as opool,
        ):
            # PE warmup on a zeroed scratch region while the first DMAs land,
            # so the frequency ramp happens off the critical path.
            warm_x = wpool.tile([P, P], f16, tag="warmx")
            nc.vector.memset(warm_x[:], 0.0)
            warm_w = wpool.tile([P, N_EXPERTS], f16, tag="warmw")
            nc.vector.memset(warm_w[:], 0.0)
            warm_ps = psum_pool.tile([P, N_EXPERTS], f32, tag="warmps")
            n_warm = 28
            for wu in range(n_warm):
                nc.tensor.matmul(
                    warm_ps[:],
                    warm_x[:],
                    warm_w[:],
                    start=(wu == 0),
                    stop=(wu == n_warm - 1),
                )

            # Weights stream first on BOTH rings concurrently, ahead of the x
            # halves, so the GEMM never weight-stalls.
            w_pieces = [(0, 28, nc.scalar), (28, 28, nc.sync)]
            wt_flat = wt.rearrange("p c e -> p (c e)")
            w_tiles = []  # (start_chunk, n_chunks, tile)
            for ws_i, (cc0, n_cc, ring) in enumerate(w_pieces):
                wtile = wpool.tile([P, n_cc * N_EXPERTS], f16, tag=f"w{ws_i}")
                ring.dma_start(
                    wtile[:],
                    wt_flat[:, cc0 * N_EXPERTS : (cc0 + n_cc) * N_EXPERTS],
                )
                w_tiles.append((cc0, n_cc, wtile))
            bias_sb = wpool.tile([P, N_EXPERTS], f32)
            nc.sync.dma_start(bias_sb[:], bb)
            oi_sb = opool.tile([P, ntiles * TOP_K], u32)
            ow_sb = opool.tile([P, ntiles * TOP_K], f32)

            def wsl(cc):  # weight AP for chunk cc
                for start, n_cc, wtile in w_tiles:
                    if start <= cc < start + n_cc:
                        return wtile[:, bass.ts(cc - start, N_EXPERTS)]
                raise AssertionError(cc)

            oi_dram = oi.rearrange("(t p) k -> p t k", p=P)
            ow_dram = ow.rearrange("(t p) k -> p t k", p=P)

            def select_chain(tt, ps):
                # scores = sigmoid(logits)
                scores = spool.tile([P, N_EXPERTS], f32, tag="scores")
                nc.scalar.activation(
                    scores[:], ps[:], mybir.ActivationFunctionType.Sigmoid
                )
                # corrected scores for selection
                scorr = spool.tile([P, N_EXPERTS], f32, tag="scorr")
                nc.vector.tensor_tensor(
                    scorr[:], scores[:], bias_sb[:], op=mybir.AluOpType.add
                )

                s3 = scorr[:].rearrange("p (g e) -> p g e", g=N_GROUP)
                # top-2-sum per group of 32 via one max8 per group
                g2 = small.tile([P, N_GROUP * 8], f32, tag="g2")
                for g in range(N_GROUP):
                    nc.vector.max(
                        g2[:, 8 * g : 8 * g + 8],
                        scorr[:, PER_GROUP * g : PER_GROUP * (g + 1)],
                    )
                g2v = g2[:].rearrange("p (g k) -> p g k", k=8)
                gs = small.tile([P, N_GROUP], f32, tag="gs")
                nc.vector.tensor_tensor(
                    gs[:].unsqueeze(2),
                    g2v[:, :, 0:1],
                    g2v[:, :, 1:2],
                    op=mybir.AluOpType.add,
                )

                # top-4 groups -> 0/1 expert mask (f32)
                g8 = small.tile([P, 8], f32, tag="g8")
                nc.vector.max(g8[:], gs[:])
                gmask = small.tile([P, N_GROUP], f32, tag="gmask")
                nc.vector.tensor_scalar(
                    gmask[:],
                    gs[:],
                    g8[:, TOPK_GROUP - 1 : TOPK_GROUP],
                    None,
                    op0=mybir.AluOpType.is_ge,
                )
                gmb = gmask[:].unsqueeze(2).broadcast_to([P, N_GROUP, PER_GROUP])

                # Packed u32 ranking keys:
                #   key = (u32((mscorr + C)*7021) << 18) | u32(score*8189*32)
                # Group-masked entries get exactly key(scorr=0) | raw payload,
                # which stays strictly below any selected key (selq dominates),
                # matching the reference's zero-fill ordering. The high bits
                # rank by corrected score; bits [5,18) carry the raw-score
                # payload; bits [0,5) are guard bits because max8 reports
                # values rounded to f32's 24-bit mantissa (ulp <= 128 at 2^31),
                # perturbing the payload by at most +-2.
                mscorr = spool.tile([P, N_EXPERTS], f32, tag="mscorr")
                nc.vector.tensor_tensor(
                    mscorr[:].rearrange("p (g e) -> p g e", g=N_GROUP),
                    s3,
                    gmb,
                    op=mybir.AluOpType.mult,
                )
                selq = spool.tile([P, N_EXPERTS], u32, tag="selq")
                nc.vector.tensor_scalar(
                    selq[:],
                    mscorr[:],
                    KEY_C,
                    SEL_SCALE,
                    op0=mybir.AluOpType.add,
                    op1=mybir.AluOpType.mult,
                )
                rawq = spool.tile([P, N_EXPERTS], u32, tag="rawq")
                nc.vector.tensor_scalar(
                    rawq[:], scores[:], RAW_SCALE, None, op0=mybir.AluOpType.mult
                )
                key = spool.tile([P, N_EXPERTS], u32, tag="key")
                kinst = nc.vector.scalar_tensor_tensor(
                    key[:],
                    in0=selq[:],
                    scalar=18,
                    in1=rawq[:],
                    op0=mybir.AluOpType.logical_shift_left,
                    op1=mybir.AluOpType.bitwise_or,
                )
                _int_imm(kinst, u32)

                # top-8 keys: indices straight to the output tile, weights
                # from the payload bits (the scale cancels in normalization).
                m8k = small.tile([P, TOP_K], u32, tag="m8k")
                nc.vector.max(m8k[:], key[:])
                nc.vector.max_index(oi_sb[:, bass.ts(tt, TOP_K)], m8k[:], key[:])
                w8u = small.tile([P, TOP_K], u32, tag="w8u")
                winst = nc.vector.tensor_scalar(
                    w8u[:],
                    m8k[:],
                    14,
                    19,
                    op0=mybir.AluOpType.logical_shift_left,
                    op1=mybir.AluOpType.logical_shift_right,
                )
                _int_imm(winst, u32)
                w8f = small.tile([P, TOP_K], f32, tag="w8f")
                nc.vector.tensor_copy(w8f[:], w8u[:])
                wsum = small.tile([P, 1], f32, tag="wsum")
                nc.vector.tensor_reduce(
                    wsum[:], w8f[:], axis=mybir.AxisListType.X, op=mybir.AluOpType.add
                )
                winv = small.tile([P, 1], f32, tag="winv")
                nc.vector.reciprocal(winv[:], wsum[:])
                nc.vector.tensor_scalar(
                    ow_sb[:, bass.ts(tt, TOP_K)],
                    w8f[:],
                    winv[:, 0:1],
                    float(ROUTED_SCALING),
                    op0=mybir.AluOpType.mult,
                    op1=mybir.AluOpType.mult,
                )
                # Store this tile's outputs (token-major [tokens, 8] in DRAM)
                # so the program tail only waits on the last tile's store.
                nc.sync.dma_start(
                    oi_dram[:, tt : tt + 1, :],
                    oi_sb[:, bass.ts(tt, TOP_K)]
                    .rearrange("p (t k) -> p t k", k=TOP_K)
                    .bitcast(i32),
                )
                nc.sync.dma_start(
                    ow_dram[:, tt : tt + 1, :],
                    ow_sb[:, bass.ts(tt, TOP_K)].rearrange(
                        "p (t k) -> p t k", k=TOP_K
                    ),
                )

            half = (KC // 2) * P
            HK = KC // 2  # 28
            live = {}
            for tt in range(ntiles + 1):
                if tt < ntiles:
                    # Load hidden tile (contiguous, 1.84 MB): halves on two
                    # rings, then issue the first-half matmuls (phase A).
                    x_tile = xpool.tile([P, KC * P], f16)
                    x_src = xt[tt].rearrange("p c t -> p (c t)")
                    nc.scalar.dma_start(x_tile[:, :half], x_src[:, :half])
                    nc.sync.dma_start(x_tile[:, half:], x_src[:, half:])
                    ps = psum_pool.tile([P, N_EXPERTS], f32, tag="ps")
                    for cc in range(HK):
                        nc.tensor.matmul(
                            ps[:],
                            x_tile[:, bass.ts(cc, P)],
                            wsl(cc),
                            start=(cc == 0),
                            stop=False,
                        )
                    live[tt] = (ps, x_tile)
                if tt == 0:
                    continue
                # Phase B (second K half) for the previous tile + selection.
                ps_b, x_b = live.pop(tt - 1)
                for cc in range(HK, KC):
                    nc.tensor.matmul(
                        ps_b[:],
                        x_b[:, bass.ts(cc, P)],
                        wsl(cc),
                        start=False,
                        stop=(cc == KC - 1),
                    )
                select_chain(tt - 1, ps_b)

    _split_excess_waits(nc)
    return nc


def _fp16_ftz(a32):
    """fp32 -> fp16 with subnormals flushed to zero (matches PE behavior)."""
    h = a32.astype(np.float16)
    h[np.abs(h) < FP16_MIN_NORMAL] = np.float16(0.0)
    return h


def prep_inputs(hidden_states, weight, e_score_correction_bias, n_cores=N_CORES):
    """Host-side shard + re-layout. Returns per-core input maps."""
    hidden_states = np.ascontiguousarray(hidden_states, dtype=np.float32)
    weight = np.ascontiguousarray(weight, dtype=np.float32)
    bias = np.asarray(e_score_correction_bias, dtype=np.float32)

    tokens = hidden_states.shape[0]
    ntiles_total = tokens // P
    tiles_per_core = ntiles_total // n_cores

    bb = np.ascontiguousarray(np.broadcast_to(bias, (P, N_EXPERTS)))

    # [T, H] -> [ntiles, t, c, p] view -> [ntiles, p, c, t]
    xh = _fp16_ftz(hidden_states)
    xt_all = xh.reshape(ntiles_total, P, KC, P).transpose(0, 3, 2, 1)
    wh = _fp16_ftz(weight)
    wt = np.ascontiguousarray(wh.T).reshape(KC, P, N_EXPERTS).transpose(1, 0, 2)
    wt = np.ascontiguousarray(wt)  # [p, c, e]

    in_maps = []
    for c in range(n_cores):
        xt_core = np.ascontiguousarray(
            xt_all[c * tiles_per_core : (c + 1) * tiles_per_core]
        )
        in_maps.append({"xt": xt_core, "wt": wt, "bb": bb})
    return in_maps, tiles_per_core * P


_PROGRAM_CACHE = {}


def run(hidden_states, weight, e_score_correction_bias, trace=False):
    in_maps, tokens_per_core = prep_inputs(
        hidden_states, weight, e_score_correction_bias
    )
    if tokens_per_core not in _PROGRAM_CACHE:
        _PROGRAM_CACHE[tokens_per_core] = build_program(tokens_per_core)
    nc = _PROGRAM_CACHE[tokens_per_core]
    res = run_bass_kernel_spmd(nc, in_maps, list(range(N_CORES)), trace=trace)
    idx = np.concatenate([res.results[i]["oi"] for i in range(N_CORES)], axis=0)
    wts = np.concatenate([res.results[i]["ow"] for i in range(N_CORES)], axis=0)
    return (idx, wts), res


def kernel(hidden_states, weight, e_score_correction_bias):
    (idx, wts), _ = run(hidden_states, weight, e_score_correction_bias)
    return idx.astype(np.int32), wts.astype(np.float32)


# revision 35
# speedup vs baseline: 1.1823x; 1.0118x over previous
"""DeepseekV3 top-k router kernel for Trainium2 (8 NeuronCores, SPMD over tokens).

Strategy: data-parallel over the token dim (16384 tokens -> 2048/core).
Per core: router GEMM as a single fp16 matmul per 128-K chunk (fp32 PSUM
accumulate; the 2e-2 error gate leaves plenty of room for fp16 rounding),
sigmoid on ScalarE. Selection packs (quantized corrected score | raw-score
payload) into u32 keys so one max8/max_index pair yields both the top-8
indices and weights; group top-2 sums come from per-group max8.
GEMM is issued in two phases per tile (first/second half of K) with phase B
lagging one tile, so PE always has scalar-ring data to chew while the sync
ring catches up.
"""

import numpy as np

import concourse.bass as bass
import concourse.mybir as mybir
import concourse.tile as tile
from concourse.bass_utils import run_bass_kernel_spmd

# Problem constants (hardcoded per contract).
TOP_K = 8
N_EXPERTS = 256
N_GROUP = 8
PER_GROUP = N_EXPERTS // N_GROUP  # 32
TOPK_GROUP = 4
ROUTED_SCALING = 2.5
HIDDEN = 7168
TOKENS = 16384
N_CORES = 8
P = 128  # partitions / tokens per tile
KC = HIDDEN // P  # 56 contraction chunks
FP16_MIN_NORMAL = 6.104e-5
KEY_C = 0.0625  # offset keeping (masked_scorr + C) positive for u32 keys
SEL_SCALE = 7021.0  # 13-bit selection grid: (1.104 + C) * 7021 < 8192
RAW_SCALE = 262048.0  # 8189 * 32: payload pre-shifted into bits [5, 18)

f32 = mybir.dt.float32
f16 = mybir.dt.float16
u32 = mybir.dt.uint32
i32 = mybir.dt.int32


def _int_imm(inst, dtype):
    """Retype float immediates of a lowered instruction as integers (walrus
    requires bitvec-op immediates to be integer-typed and match src/dst)."""
    for arg in inst.ins.ins:
        if isinstance(arg, mybir.ImmediateValue):
            arg.dtype = dtype
            arg.value = int(arg.value)
    return inst


# walrus in this toolchain rejects more than one sync-wait per instruction.
# Post-pass: move excess waits onto same-engine NOPs inserted just before the
# offending instruction (engine stalls on the NOPs first — semantics preserved).
_MAX_WAITS = 1


def _split_excess_waits(nc, max_waits=_MAX_WAITS):
    all_bbs = [bb for fn in nc.m.functions for bb in fn.blocks]
    pre_by_name = {}
    appended = set()
    for bb in all_bbs:
        for inst in bb.instructions:
            si = inst.sync_info
            if si is None:
                continue
            waits = list(si.on_wait or [])
            if len(waits) <= max_waits:
                continue
            if inst.engine not in nc.engines:
                continue
            eng = nc.engines[inst.engine]
            n_extra = len(waits) - max_waits
            pre = []
            for j in range(0, n_extra, max_waits):
                nb = eng.nop(nofuse=True)
                nb.ins.sync_info = mybir.SyncInfo(
                    on_wait=waits[j : j + max_waits], on_update=[]
                )
                pre.append(nb.ins)
                appended.add(nb.ins.name)
            si.on_wait = waits[n_extra:]
            inst.sync_info = si
            pre_by_name[inst.name] = pre
    if not pre_by_name:
        return
    for bb in all_bbs:
        rebuilt = []
        changed = False
        for inst in bb.instructions:
            if inst.name in appended:
                changed = True
                continue
            if inst.name in pre_by_name:
                rebuilt.extend(pre_by_name[inst.name])
                changed = True
            rebuilt.append(inst)
        if changed:
            bb.instructions = rebuilt


def build_program(tokens_per_core: int):
    """Build the single-core Bass program (same program runs SPMD on all cores)."""
    ntiles = tokens_per_core // P
    nc = bass.Bass("TRN2", target_bir_lowering=False, debug=False)

    # Host-prepared layouts (see prep_inputs):
    #  xt [ntiles, 128(p), 56(c), 128(t)] f16 ; wt [128(p), 56(c), 256(e)] f16
    #  bb [128, 256] f32 (bias row-broadcast)
    xt = nc.dram_tensor("xt", [ntiles, P, KC, P], f16, kind="ExternalInput").ap()
    wt = nc.dram_tensor("wt", [P, KC, N_EXPERTS], f16, kind="ExternalInput").ap()
    bb = nc.dram_tensor("bb", [P, N_EXPERTS], f32, kind="ExternalInput").ap()
    oi = nc.dram_tensor("oi", [tokens_per_core, TOP_K], i32, kind="ExternalOutput").ap()
    ow = nc.dram_tensor("ow", [tokens_per_core, TOP_K], f32, kind="ExternalOutput").ap()

    with tile.TileContext(nc) as tc:
        with (
            tc.tile_pool(name="wpool", bufs=1) as wpool,
            tc.tile_pool(name="xpool", bufs=5) as xpool,
            tc.tile_pool(name="psum", bufs=3, space="PSUM") as psum_pool,
            tc.tile_pool(name="spool", bufs=2) as spool,
            tc.tile_pool(name="small", bufs=2) as small,
            tc.tile_pool(name="opool", bufs=1) as opool,
        ):
            # PE warmup on a zeroed scratch region while the first DMAs land,
            # so the frequency ramp happens off the critical path.
            warm_x = wpool.tile([P, P], f16, tag="warmx")
            nc.vector.memset(warm_x[:], 0.0)
            warm_w = wpool.tile([P, N_EXPERTS], f16, tag="warmw")
            nc.vector.memset(warm_w[:], 0.0)
            warm_ps = psum_pool.tile([P, N_EXPERTS], f32, tag="warmps")
            n_warm = 28
            for wu in range(n_warm):
                nc.tensor.matmul(
                    warm_ps[:],
                    warm_x[:],
                    warm_w[:],
                    start=(wu == 0),
                    stop=(wu == n_warm - 1),
                )

            # Weights stream first on BOTH rings concurrently, ahead of the x
            # halves, so the GEMM never weight-stalls.
            w_pieces = [(0, 28, nc.scalar), (28, 28, nc.sync)]
            wt_flat = wt.rearrange("p c e -> p (c e)")
            w_tiles = []  # (start_chunk, n_chunks, tile)
            for ws_i, (cc0, n_cc, ring) in enumerate(w_pieces):
                wtile = wpool.tile([P, n_cc * N_EXPERTS], f16, tag=f"w{ws_i}")
                ring.dma_start(
                    wtile[:],
                    wt_flat[:, cc0 * N_EXPERTS : (cc0 + n_cc) * N_EXPERTS],
                )
                w_tiles.append((cc0, n_cc, wtile))
            bias_sb = wpool.tile([P, N_EXPERTS], f32)
            nc.sync.dma_start(bias_sb[:], bb)
            oi_sb = opool.tile([P, ntiles * TOP_K], u32)
            ow_sb = opool.tile([P, ntiles * TOP_K], f32)

            def wsl(cc):  # weight AP for chunk cc
                for start, n_cc, wtile in w_tiles:
                    if start <= cc < start + n_cc:
                        return wtile[:, bass.ts(cc - start, N_EXPERTS)]
                raise AssertionError(cc)

            oi_dram = oi.rearrange("(t p) k -> p t k", p=P)
            ow_dram = ow.rearrange("(t p) k -> p t k", p=P)

            def select_chain(tt, ps):
                # scores = sigmoid(logits)
                scores = spool.tile([P, N_EXPERTS], f32, tag="scores")
                nc.scalar.activation(
                    scores[:], ps[:], mybir.ActivationFunctionType.Sigmoid
                )
                # corrected scores for selection
                scorr = spool.tile([P, N_EXPERTS], f32, tag="scorr")
                nc.vector.tensor_tensor(
                    scorr[:], scores[:], bias_sb[:], op=mybir.AluOpType.add
                )

                s3 = scorr[:].rearrange("p (g e) -> p g e", g=N_GROUP)
                # top-2-sum per group of 32 via one max8 per group
                g2 = small.tile([P, N_GROUP * 8], f32, tag="g2")
                for g in range(N_GROUP):
                    nc.vector.max(
                        g2[:, 8 * g : 8 * g + 8],
                        scorr[:, PER_GROUP * g : PER_GROUP * (g + 1)],
                    )
                g2v = g2[:].rearrange("p (g k) -> p g k", k=8)
                gs = small.tile([P, N_GROUP], f32, tag="gs")
                nc.vector.tensor_tensor(
                    gs[:].unsqueeze(2),
                    g2v[:, :, 0:1],
                    g2v[:, :, 1:2],
                    op=mybir.AluOpType.add,
                )

                # top-4 groups -> 0/1 expert mask (f32)
                g8 = small.tile([P, 8], f32, tag="g8")
                nc.vector.max(g8[:], gs[:])
                gmask = small.tile([P, N_GROUP], f32, tag="gmask")
                nc.vector.tensor_scalar(
                    gmask[:],
                    gs[:],
                    g8[:, TOPK_GROUP - 1 : TOPK_GROUP],
                    None,
                    op0=mybir.AluOpType.is_ge,
                )
                gmb = gmask[:].unsqueeze(2).broadcast_to([P, N_GROUP, PER_GROUP])

                # Packed u32 ranking keys:
                #   key = (u32((mscorr + C)*7021) << 18) | u32(score*8189*32)
                # Group-masked entries get exactly key(scorr=0) | raw payload,
                # which stays strictly below any selected key (selq dominates),
                # matching the reference's zero-fill ordering. The high bits
                # rank by corrected score; bits [5,18) carry the raw-score
                # payload; bits [0,5) are guard bits because max8 reports
                # values rounded to f32's 24-bit mantissa (ulp <= 128 at 2^31),
                # perturbing the payload by at most +-2.
                mscorr = spool.tile([P, N_EXPERTS], f32, tag="mscorr")
                nc.vector.tensor_tensor(
                    mscorr[:].rearrange("p (g e) -> p g e", g=N_GROUP),
                    s3,
                    gmb,
                    op=mybir.AluOpType.mult,
                )
                selq = spool.tile([P, N_EXPERTS], u32, tag="selq")
                nc.vector.tensor_scalar(
                    selq[:],
                    mscorr[:],
                    KEY_C,
                    SEL_SCALE,
                    op0=mybir.AluOpType.add,
                    op1=mybir.AluOpType.mult,
                )
                rawq = spool.tile([P, N_EXPERTS], u32, tag="rawq")
                nc.vector.tensor_scalar(
                    rawq[:], scores[:], RAW_SCALE, None, op0=mybir.AluOpType.mult
                )
                key = spool.tile([P, N_EXPERTS], u32, tag="key")
                kinst = nc.vector.scalar_tensor_tensor(
                    key[:],
                    in0=selq[:],
                    scalar=18,
                    in1=rawq[:],
                    op0=mybir.AluOpType.logical_shift_left,
                    op1=mybir.AluOpType.bitwise_or,
                )
                _int_imm(kinst, u32)

                # top-8 keys: indices straight to the output tile, weights
                # from the payload bits (the scale cancels in normalization).
                m8k = small.tile([P, TOP_K], u32, tag="m8k")
                nc.vector.max(m8k[:], key[:])
                nc.vector.max_index(oi_sb[:, bass.ts(tt, TOP_K)], m8k[:], key[:])
                w8u = small.tile([P, TOP_K], u32, tag="w8u")
                winst = nc.vector.tensor_scalar(
                    w8u[:],
                    m8k[:],
                    14,
                    19,
                    op0=mybir.AluOpType.logical_shift_left,
                    op1=mybir.AluOpType.logical_shift_right,
                )
                _int_imm(winst, u32)
                w8f = small.tile([P, TOP_K], f32, tag="w8f")
                nc.vector.tensor_copy(w8f[:], w8u[:])
                wsum = small.tile([P, 1], f32, tag="wsum")
                nc.vector.tensor_reduce(
                    wsum[:], w8f[:], axis=mybir.AxisListType.X, op=mybir.AluOpType.add
                )
                winv = small.tile([P, 1], f32, tag="winv")
                nc.vector.reciprocal(winv[:], wsum[:])
                nc.vector.tensor_scalar(
                    ow_sb[:, bass.ts(tt, TOP_K)],
                    w8f[:],
                    winv[:, 0:1],
                    float(ROUTED_SCALING),
                    op0=mybir.AluOpType.mult,
                    op1=mybir.AluOpType.mult,
                )
                # Store this tile's outputs (token-major [tokens, 8] in DRAM)
                # so the program tail only waits on the last tile's store.
                nc.sync.dma_start(
                    oi_dram[:, tt : tt + 1, :],
                    oi_sb[:, bass.ts(tt, TOP_K)]
                    .rearrange("p (t k) -> p t k", k=TOP_K)
                    .bitcast(i32),
                )
                nc.sync.dma_start(
                    ow_dram[:, tt : tt + 1, :],
                    ow_sb[:, bass.ts(tt, TOP_K)].rearrange(
                        "p (t k) -> p t k", k=TOP_K
                    ),
                )

            half = (KC // 2) * P
            HK = KC // 2  # 28
            live = {}
            for tt in range(ntiles + 1):
                if tt < ntiles:
                    # Load hidden tile (contiguous, 1.84 MB): halves on two
                    # rings, then issue the first-half matmuls (phase A).
                    x_tile = xpool.tile([P, KC * P], f16)
                    x_src = xt[tt].rearrange("p c t -> p (c t)")
                    nc.scalar.dma_start(x_tile[:, :half], x_src[:, :half])
                    nc.sync.dma_start(x_tile[:, half:], x_src[:, half:])
                    ps = psum_pool.tile([P, N_EXPERTS], f32, tag="ps")
                    for cc in range(HK):
                        nc.tensor.matmul(
                            ps[:],
                            x_tile[:, bass.ts(cc, P)],
                            wsl(cc),
                            start=(cc == 0),
                            stop=False,
                        )
                    live[tt] = (ps, x_tile)
                if tt == 0:
                    continue
                # Phase B (second K half) for the previous tile + selection.
                ps_b, x_b = live.pop(tt - 1)
                for cc in range(HK, KC):
                    nc.tensor.matmul(
                        ps_b[:],
                        x_b[:, bass.ts(cc, P)],
                        wsl(cc),
                        start=False,
                        stop=(cc == KC - 1),
                    )
                select_chain(tt - 1, ps_b)

    _split_excess_waits(nc)
    return nc


def _fp16_ftz(a32):
    """fp32 -> fp16 with subnormals flushed to zero (matches PE behavior)."""
    h = a32.astype(np.float16)
    h[np.abs(h) < FP16_MIN_NORMAL] = np.float16(0.0)
    return h


def prep_inputs(hidden_states, weight, e_score_correction_bias, n_cores=N_CORES):
    """Host-side shard + re-layout. Returns per-core input maps."""
    hidden_states = np.ascontiguousarray(hidden_states, dtype=np.float32)
    weight = np.ascontiguousarray(weight, dtype=np.float32)
    bias = np.asarray(e_score_correction_bias, dtype=np.float32)

    tokens = hidden_states.shape[0]
    ntiles_total = tokens // P
    tiles_per_core = ntiles_total // n_cores

    bb = np.ascontiguousarray(np.broadcast_to(bias, (P, N_EXPERTS)))

    # [T, H] -> [ntiles, t, c, p] view -> [ntiles, p, c, t]
    xh = _fp16_ftz(hidden_states)
    xt_all = xh.reshape(ntiles_total, P, KC, P).transpose(0, 3, 2, 1)
    wh = _fp16_ftz(weight)
    wt = np.ascontiguousarray(wh.T).reshape(KC, P, N_EXPERTS).transpose(1, 0, 2)
    wt = np.ascontiguousarray(wt)  # [p, c, e]

    in_maps = []
    for c in range(n_cores):
        xt_core = np.ascontiguousarray(
            xt_all[c * tiles_per_core : (c + 1) * tiles_per_core]
        )
        in_maps.append({"xt": xt_core, "wt": wt, "bb": bb})
    return in_maps, tiles_per_core * P


_PROGRAM_CACHE = {}


def run(hidden_states, weight, e_score_correction_bias, trace=False):
    in_maps, tokens_per_core = prep_inputs(
        hidden_states, weight, e_score_correction_bias
    )
    if tokens_per_core not in _PROGRAM_CACHE:
        _PROGRAM_CACHE[tokens_per_core] = build_program(tokens_per_core)
    nc = _PROGRAM_CACHE[tokens_per_core]
    res = run_bass_kernel_spmd(nc, in_maps, list(range(N_CORES)), trace=trace)
    idx = np.concatenate([res.results[i]["oi"] for i in range(N_CORES)], axis=0)
    wts = np.concatenate([res.results[i]["ow"] for i in range(N_CORES)], axis=0)
    return (idx, wts), res


def kernel(hidden_states, weight, e_score_correction_bias):
    (idx, wts), _ = run(hidden_states, weight, e_score_correction_bias)
    return idx.astype(np.int32), wts.astype(np.float32)
